# revision 23
# baseline (speedup 1.0000x reference)
"""Causal self-attention (B=2, N=2048, D=1024, H=16) on 8 trn2 NeuronCores.

Sharding: data-parallel over batch (2) x tensor-parallel over heads (4 head
groups of 4 heads) = 8 cores. Each core computes QKV projection for its 4
heads, causal attention, and its partial of the output projection (contraction
over its heads' dims). The host sums the 4 partials per batch element and adds
the constant term (out bias + v-bias routed through W_out, exact because
softmax rows sum to 1).

Per-core kernel (all matmuls in float32r = full PE rate at free dim >= 256):
  - q,k kept transposed [head_dim, tokens]; v kept natural [tokens, head_dim]
    with a ones column per head so the p@v matmul also produces the softmax
    denominators.
  - scores computed transposed st[j,i] = k_j . q_i for 128-row j-tiles and
    512-col i-chunks; two heads packed side by side into one [128,1024] PSUM
    tile via PE row-group packing (K=64 each, concurrent).
  - exp on ACT with scale=1/8 straight out of PSUM (no max subtraction; the
    score range for these inputs is a few units). Causal masking only on the
    4 diagonal blocks per chunk via gpsimd affine_select.
  - softmax normalization deferred past p@v using the ones-column sums.
  - emission interleaves next-chunk QKV and prev-chunk out-projection matmuls
    into the attention block stream so the PE never idles long enough for the
    HAM clock gate to re-throttle.
"""

import sys
from collections import deque

if '/opt/trn_rl_repo' not in sys.path:
    sys.path.insert(0, '/opt/trn_rl_repo')

import numpy as np

import concourse.bacc as bacc
import concourse.mybir as mybir
import concourse.tile as tile
from concourse.bass_utils import run_bass_kernel_spmd

F32 = mybir.dt.float32
F32R = mybir.dt.float32r
EXP = mybir.ActivationFunctionType.Exp
MULT = mybir.AluOpType.mult
ADD = mybir.AluOpType.add
IS_GE = mybir.AluOpType.is_ge

B, N, D, H = 2, 2048, 1024, 16
HD = D // H          # 64
HPC = 4              # heads per core
NCORES = 8
NT = N // 512        # 4 token chunks of 512
NJ = N // 128        # 16 key tiles of 128
SCALE = HD ** (-0.5)

USE_RECIP_FAST = True
ATT_BF16 = False   # bf16 for q/k/v/p attention tiles (faster PE, ~5x larger rounding)
BF16 = mybir.dt.bfloat16


def _emit(ctx, nc, tc, dram):
    xT, wqk, wv, wo, bqk, y = (
        dram['xT'], dram['wqk'], dram['wv'], dram['wo'], dram['bqk'],
        dram['y'])

    cp = ctx.enter_context(tc.tile_pool(name="const", bufs=1))
    xp = ctx.enter_context(tc.tile_pool(name="xwin", bufs=24))
    pp = ctx.enter_context(tc.tile_pool(name="pexp", bufs=6))
    sm = ctx.enter_context(tc.tile_pool(name="small", bufs=4))
    psc = ctx.enter_context(tc.tile_pool(name="psc", bufs=2, space="PSUM"))
    pao = ctx.enter_context(tc.tile_pool(name="pao", bufs=2, space="PSUM"))
    pms = ctx.enter_context(tc.tile_pool(name="pms", bufs=2, space="PSUM"))

    # ---- persistent tiles -------------------------------------------------
    wqk_t = [cp.tile([128, 512], F32R, tag=f"wqk{f}", name=f"wqk{f}")
             for f in range(8)]
    wv_t = [cp.tile([128, 256], F32R, tag=f"wv{f}", name=f"wv{f}")
            for f in range(8)]
    wo_t = [cp.tile([128, 1024], F32R, tag=f"wo{k}", name=f"wo{k}")
            for k in range(2)]
    bqk_t = [cp.tile([128, 1], F32, tag=f"bqk{r}", name=f"bqk{r}")
             for r in range(4)]
    # q,k transposed: 4 tiles [128 dims, 2048 tokens]; rt 0,1 = q; rt 2,3 = k
    att_dt = BF16 if ATT_BF16 else F32R
    qkT = [cp.tile([128, N], att_dt, tag=f"qkT{r}", name=f"qkT{r}")
           for r in range(4)]
    # v natural per j-tile with ones columns: [v_h0|1|v_h1|1|v_h2|1|v_h3|1]
    v_t = [cp.tile([128, 65 * HPC], att_dt, tag=f"v{j}", name=f"v{j}")
           for j in range(NJ)]
    # normalized attention output, transposed [head dims, tokens]
    aoT = [cp.tile([128, N], F32R, tag=f"aoT{k}", name=f"aoT{k}")
           for k in range(2)]

    # weights on the (otherwise idle at startup) gpsimd SWDGE queue so the
    # x-window loads on the sync queue aren't stuck behind them
    for f in range(8):
        nc.gpsimd.dma_start(out=wqk_t[f][:],
                            in_=wqk[f * 128:(f + 1) * 128, :])
    for f in range(8):
        nc.gpsimd.dma_start(out=wv_t[f][:], in_=wv[f * 128:(f + 1) * 128, :])
    for k in range(2):
        nc.gpsimd.dma_start(out=wo_t[k][:], in_=wo[k * 128:(k + 1) * 128, :])
    for r in range(4):
        nc.gpsimd.dma_start(out=bqk_t[r][:],
                            in_=bqk[r * 128:(r + 1) * 128, :])

    # ---- task generators (each yielded thunk emits ~one PE instruction) ---
    def qkv_tasks(c, xw):
        # q,k transposed r-tiles
        for rt in range(4):
            st = {}
            def _mk(rt, f, st):
                def _t():
                    if f == 0:
                        st['ps'] = pms.tile([128, 512], F32, tag="ms",
                                            name=f"qk{c}_{rt}")
                    nc.tensor.matmul(st['ps'][:],
                                     wqk_t[f][:, rt * 128:(rt + 1) * 128],
                                     xw[f][:], start=(f == 0), stop=(f == 7))
                    if f == 7:
                        nc.vector.tensor_scalar_add(
                            qkT[rt][:, c * 512:(c + 1) * 512], st['ps'][:],
                            bqk_t[rt][:])
                return _t
            for f in range(8):
                yield _mk(rt, f, st)
        # v natural t-tiles
        for tt in range(4):
            jt = 4 * c + tt
            st = {}
            def _mk(tt, jt, f, st):
                def _t():
                    if f == 0:
                        st['ps'] = pms.tile([128, 256], F32, tag="ms",
                                            name=f"v{c}_{tt}")
                    nc.tensor.matmul(st['ps'][:],
                                     xw[f][:, tt * 128:(tt + 1) * 128],
                                     wv_t[f][:], start=(f == 0), stop=(f == 7))
                    if f == 7:
                        ps = st['ps']
                        v3 = v_t[jt][:].rearrange("p (g e) -> p g e", e=65)
                        nc.vector.tensor_scalar(
                            out=v3[:, :, 64:65],
                            in0=ps[:, 0:4].rearrange("p (g e) -> p g e", e=1),
                            scalar1=0.0, scalar2=1.0, op0=MULT, op1=ADD)
                        nc.vector.tensor_copy(
                            out=v3[:, :, 0:64],
                            in_=ps[:].rearrange("p (g e) -> p g e", e=64))
                return _t
            for f in range(8):
                yield _mk(tt, jt, f, st)

    def yproj_tasks(c):
        for tt in range(4):
            t0 = c * 512 + tt * 128
            st = {}
            for ec in range(2):
                def _mk(t0, ec, k, st):
                    def _t():
                        if ec == 0 and k == 0:
                            st['y'] = sm.tile([128, 1024], F32, tag="y",
                                              name=f"y{t0}", bufs=3)
                        if k == 0:
                            st['ps'] = pms.tile([128, 512], F32, tag="ms",
                                                name=f"yp{t0}_{ec}")
                        nc.tensor.matmul(
                            st['ps'][:], aoT[k][:, t0:t0 + 128],
                            wo_t[k][:, ec * 512:(ec + 1) * 512],
                            start=(k == 0), stop=(k == 1))
                        if k == 1:
                            nc.vector.tensor_copy(
                                out=st['y'][:, ec * 512:(ec + 1) * 512],
                                in_=st['ps'][:])
                            if ec == 1:
                                nc.sync.dma_start(out=y[t0:t0 + 128, :],
                                                  in_=st['y'][:])
                    return _t
                for k in range(2):
                    yield _mk(t0, ec, k, st)

    def start_x_window(c):
        xw = [xp.tile([128, 512], F32R, tag="xw", name=f"xw{c}_{f}")
              for f in range(8)]
        for f in range(8):
            nc.sync.dma_start(
                out=xw[f][:],
                in_=xT[f * 128:(f + 1) * 128, c * 512:(c + 1) * 512])
        return xw

    def emit_pv(pr, jt, p_entry, ao_e, ao_o, first, last):
        p, i0 = p_entry
        for g, ao in ((2 * pr, ao_e), (2 * pr + 1, ao_o)):
            nc.tensor.matmul(
                ao[:, i0:512], v_t[jt][:, g * 65:g * 65 + 65],
                p[:, (g % 2) * 512 + i0:(g % 2) * 512 + 512],
                start=first, stop=last)

    # ---- main schedule ----------------------------------------------------
    rscratch_t = nc.dram_tensor("rscratch", [16, 1, 512], F32)
    rscratch = [rscratch_t.ap()[i] for i in range(16)]
    fill = deque()
    xw0 = start_x_window(0)
    for t in qkv_tasks(0, xw0):
        t()

    for c in range(NT):
        if c + 1 < NT:
            xw_next = start_x_window(c + 1)
            fill.extend(qkv_tasks(c + 1, xw_next))
        if c >= 1:
            fill.extend(yproj_tasks(c - 1))

        njt = 4 * c + 4
        nblocks = 2 * njt
        blk = 0

        def drain_fillers(blocks_left):
            want = -(-len(fill) // max(blocks_left, 1))  # ceil
            for _ in range(min(want, len(fill))):
                fill.popleft()()

        for pr in range(2):
            qt, kt = qkT[pr], qkT[2 + pr]
            ao_e = pao.tile([65, 512], F32, tag="ao", name=f"aoe{c}_{pr}")
            ao_o = pao.tile([65, 512], F32, tag="ao", name=f"aoo{c}_{pr}")
            plist = []
            for jt in range(njt):
                # diagonal blocks: columns i < i0 are fully masked, skip them
                d = jt - 4 * c
                i0 = 128 * d if d >= 1 else 0
                w = 512 - i0
                sc = psc.tile([128, 1024], F32, tag="sc",
                              name=f"sc{c}_{pr}_{jt}")
                nc.tensor.matmul(sc[:, i0:512],
                                 kt[0:64, jt * 128:(jt + 1) * 128],
                                 qt[0:64, c * 512 + i0:(c + 1) * 512],
                                 start=True, stop=True)
                nc.tensor.matmul(sc[:, 512 + i0:1024],
                                 kt[64:128, jt * 128:(jt + 1) * 128],
                                 qt[64:128, c * 512 + i0:(c + 1) * 512],
                                 start=True, stop=True)
                p = pp.tile([128, 1024], att_dt, tag="p", name=f"p{c}_{pr}_{jt}")
                p3 = p[:].rearrange("p (h i) -> p h i", i=512)[:, :, i0:512]
                sc3 = sc[:].rearrange("p (h i) -> p h i", i=512)[:, :, i0:512]
                nc.scalar.activation(p3, sc3, EXP, scale=SCALE)
                if d >= 0:
                    nc.gpsimd.affine_select(
                        out=p3, in_=p3, compare_op=IS_GE, fill=0.0,
                        base=0, channel_multiplier=-1,
                        pattern=[[0, 2], [1, w]])
                plist.append((p, i0))
                if jt >= 1:
                    emit_pv(pr, jt - 1, plist[jt - 1], ao_e, ao_o,
                            first=(jt - 1 == 0), last=False)
                blk += 1
                drain_fillers(nblocks - blk)
            emit_pv(pr, njt - 1, plist[njt - 1], ao_e, ao_o,
                    first=(njt == 1), last=True)

            # normalization: copy PSUM out fast (frees the ao banks); the
            # reciprocal+broadcast+mul tail is deferred into the next chunk's
            # filler stream so it never gates this pipeline.
            for g, ao in ((2 * pr, ao_e), (2 * pr + 1, ao_o)):
                t = sm.tile([65, 512], F32, tag="aosb", name=f"aosb{c}_{g}")
                nc.vector.tensor_copy(out=t[:], in_=ao[:])

                def _norm_tail(c=c, pr=pr, g=g, t=t):
                    r = sm.tile([1, 512], F32, tag="r", name=f"r{c}_{g}", bufs=2)
                    if USE_RECIP_FAST:
                        # custom-DVE ops don't handle nonzero partition
                        # offsets; stage the sums row at partition 0 first.
                        s_row = sm.tile([1, 512], F32, tag="srow",
                                        name=f"srow{c}_{g}", bufs=2)
                        nc.vector.tensor_copy(out=s_row[:], in_=t[64:65, :])
                        nc.vector.reciprocal_approx_fast(out=r[:],
                                                         in_=s_row[:])
                    else:
                        nc.vector.reciprocal(r[:], t[64:65, :])
                    # broadcast R across 64 partitions via a DRAM bounce
                    # (keeps gpsimd free for the causal-mask selects; an
                    # SBUF-source broadcast AP is not expressible).
                    rd = rscratch[4 * c + g]
                    nc.sync.dma_start(out=rd, in_=r[:])
                    rb = sm.tile([64, 512], F32, tag="rb", name=f"rb{c}_{g}", bufs=2)
                    nc.sync.dma_start(out=rb[:],
                                      in_=rd.to_broadcast([64, 512]))
                    nc.vector.tensor_mul(
                        aoT[pr][(g % 2) * 64:(g % 2) * 64 + 64,
                                c * 512:(c + 1) * 512],
                        t[0:64, :], rb[:])
                fill.append(_norm_tail)

    while fill:
        fill.popleft()()
    for t in yproj_tasks(NT - 1):
        t()


_CACHE = {}


def _build():
    if 'nc' in _CACHE:
        return _CACHE['nc']
    nc = bacc.Bacc("TRN2", target_bir_lowering=False, debug=False)
    dram = {
        'xT': nc.dram_tensor("xT", [D, N], F32R, kind="ExternalInput").ap(),
        'wqk': nc.dram_tensor("wqk", [D, 512], F32R, kind="ExternalInput").ap(),
        'wv': nc.dram_tensor("wv", [D, 256], F32R, kind="ExternalInput").ap(),
        'wo': nc.dram_tensor("wo", [256, D], F32R, kind="ExternalInput").ap(),
        'bqk': nc.dram_tensor("bqk", [512, 1], F32, kind="ExternalInput").ap(),
        'y': nc.dram_tensor("y", [N, D], F32, kind="ExternalOutput").ap(),
    }
    from contextlib import ExitStack
    with tile.TileContext(nc) as tc, ExitStack() as ctx:
        _emit(ctx, nc, tc, dram)
    nc.compile()
    _CACHE['nc'] = nc
    return nc


def _prep_core_inputs(x, W_qkv, b_qkv, W_out, core):
    b = core // 4
    h0 = HPC * (core % 4)
    r0 = HD * h0
    q_rows = W_qkv[r0:r0 + 256]
    k_rows = W_qkv[D + r0:D + r0 + 256]
    v_rows = W_qkv[2 * D + r0:2 * D + r0 + 256]
    return {
        'xT': np.ascontiguousarray(x[b].T),
        'wqk': np.ascontiguousarray(np.concatenate([q_rows, k_rows], 0).T),
        'wv': np.ascontiguousarray(v_rows.T),
        'wo': np.ascontiguousarray(W_out[:, r0:r0 + 256].T),
        'bqk': np.concatenate(
            [b_qkv[r0:r0 + 256], b_qkv[D + r0:D + r0 + 256]]).reshape(512, 1),
    }


def kernel(x, W_qkv, b_qkv, W_out, b_out, _trace=False, _tmpdir=None):
    x = np.asarray(x, dtype=np.float32)
    W_qkv = np.asarray(W_qkv, dtype=np.float32)
    b_qkv = np.asarray(b_qkv, dtype=np.float32)
    W_out = np.asarray(W_out, dtype=np.float32)
    b_out = np.asarray(b_out, dtype=np.float32)

    in_maps = [_prep_core_inputs(x, W_qkv, b_qkv, W_out, c)
               for c in range(NCORES)]
    nc = _build()
    res = run_bass_kernel_spmd(nc, in_maps, list(range(NCORES)),
                               trace=_trace, tmpdir=_tmpdir)

    # v-bias contribution (softmax rows sum to 1) + output bias, as one
    # constant vector added on the host.
    bv = b_qkv[2 * D:3 * D]
    const = (b_out.astype(np.float64)
             + W_out.astype(np.float64) @ bv.astype(np.float64))
    out = np.empty((B, N, D), dtype=np.float32)
    for b in range(B):
        acc = np.zeros((N, D), dtype=np.float64)
        for g in range(4):
            acc += res.results[4 * b + g]['y'].astype(np.float64)
        out[b] = (acc + const).astype(np.float32)
    if _trace:
        kernel.last_exec_time_ns = res.exec_time_ns
        kernel.last_trace = (res.instructions_and_trace[1]
                             if res.instructions_and_trace else None)
    return out


# revision 24
# speedup vs baseline: 1.1070x; 1.1070x over previous
"""Causal self-attention (B=2, N=2048, D=1024, H=16) on 8 trn2 NeuronCores.

Sharding: data-parallel over batch (2) x tensor-parallel over heads (4 head
groups of 4 heads) = 8 cores. Each core computes QKV projection for its 4
heads, causal attention, and its partial of the output projection (contraction
over its heads' dims). The host sums the 4 partials per batch element and adds
the constant term (out bias + v-bias routed through W_out, exact because
softmax rows sum to 1).

Per-core kernel (all matmuls in float32r = full PE rate at free dim >= 256):
  - q,k kept transposed [head_dim, tokens]; v kept natural [tokens, head_dim]
    with a ones column per head so the p@v matmul also produces the softmax
    denominators.
  - scores computed transposed st[j,i] = k_j . q_i for 128-row j-tiles and
    512-col i-chunks; two heads packed side by side into one [128,1024] PSUM
    tile via PE row-group packing (K=64 each, concurrent).
  - exp on ACT with scale=1/8 straight out of PSUM (no max subtraction; the
    score range for these inputs is a few units). Causal masking only on the
    4 diagonal blocks per chunk via gpsimd affine_select.
  - softmax normalization deferred past p@v using the ones-column sums.
  - emission interleaves next-chunk QKV and prev-chunk out-projection matmuls
    into the attention block stream so the PE never idles long enough for the
    HAM clock gate to re-throttle.
"""

import sys
from collections import deque

if '/opt/trn_rl_repo' not in sys.path:
    sys.path.insert(0, '/opt/trn_rl_repo')

import numpy as np

import concourse.bacc as bacc
import concourse.mybir as mybir
import concourse.tile as tile
from concourse.bass_utils import run_bass_kernel_spmd

F32 = mybir.dt.float32
F32R = mybir.dt.float32r
EXP = mybir.ActivationFunctionType.Exp
MULT = mybir.AluOpType.mult
ADD = mybir.AluOpType.add
IS_GE = mybir.AluOpType.is_ge

B, N, D, H = 2, 2048, 1024, 16
HD = D // H          # 64
HPC = 4              # heads per core
NCORES = 8
NT = N // 512        # 4 token chunks of 512
NJ = N // 128        # 16 key tiles of 128
SCALE = HD ** (-0.5)

USE_RECIP_FAST = True
ATT_BF16 = False   # bf16 for q/k/v/p attention tiles (faster PE, ~5x larger rounding)
BF16 = mybir.dt.bfloat16


def _emit(ctx, nc, tc, dram):
    xT, wqk, wv, wo, bqk, y = (
        dram['xT'], dram['wqk'], dram['wv'], dram['wo'], dram['bqk'],
        dram['y'])

    cp = ctx.enter_context(tc.tile_pool(name="const", bufs=1))
    xp = ctx.enter_context(tc.tile_pool(name="xwin", bufs=24))
    pp = ctx.enter_context(tc.tile_pool(name="pexp", bufs=6))
    sm = ctx.enter_context(tc.tile_pool(name="small", bufs=4))
    psc = ctx.enter_context(tc.tile_pool(name="psc", bufs=2, space="PSUM"))
    pao = ctx.enter_context(tc.tile_pool(name="pao", bufs=2, space="PSUM"))
    pms = ctx.enter_context(tc.tile_pool(name="pms", bufs=2, space="PSUM"))

    # ---- persistent tiles -------------------------------------------------
    wqk_t = [cp.tile([128, 512], F32R, tag=f"wqk{f}", name=f"wqk{f}")
             for f in range(8)]
    wv_t = [cp.tile([128, 256], F32R, tag=f"wv{f}", name=f"wv{f}")
            for f in range(8)]
    wo_t = [cp.tile([128, 1024], F32R, tag=f"wo{k}", name=f"wo{k}")
            for k in range(2)]
    bqk_t = [cp.tile([128, 1], F32, tag=f"bqk{r}", name=f"bqk{r}")
             for r in range(4)]
    # q,k transposed: 4 tiles [128 dims, 2048 tokens]; rt 0,1 = q; rt 2,3 = k
    att_dt = BF16 if ATT_BF16 else F32R
    qkT = [cp.tile([128, N], att_dt, tag=f"qkT{r}", name=f"qkT{r}")
           for r in range(4)]
    # v natural per j-tile with ones columns: [v_h0|1|v_h1|1|v_h2|1|v_h3|1]
    v_t = [cp.tile([128, 65 * HPC], att_dt, tag=f"v{j}", name=f"v{j}")
           for j in range(NJ)]
    # normalized attention output, transposed [head dims, tokens]
    aoT = [cp.tile([128, N], F32R, tag=f"aoT{k}", name=f"aoT{k}")
           for k in range(2)]

    # weights on the (otherwise idle at startup) gpsimd SWDGE queue so the
    # x-window loads on the sync queue aren't stuck behind them; biases first
    # (tiny, and the first qkT bias-add gates the whole attention pipeline)
    for r in range(4):
        nc.gpsimd.dma_start(out=bqk_t[r][:],
                            in_=bqk[r * 128:(r + 1) * 128, :])
    for f in range(8):
        nc.gpsimd.dma_start(out=wqk_t[f][:],
                            in_=wqk[f * 128:(f + 1) * 128, :])
    for f in range(8):
        nc.gpsimd.dma_start(out=wv_t[f][:], in_=wv[f * 128:(f + 1) * 128, :])
    for k in range(2):
        nc.gpsimd.dma_start(out=wo_t[k][:], in_=wo[k * 128:(k + 1) * 128, :])

    # ---- task generators (each yielded thunk emits ~one PE instruction) ---
    def qkv_tasks(c, xw):
        # q,k transposed r-tiles
        for rt in range(4):
            st = {}
            def _mk(rt, f, st):
                def _t():
                    if f == 0:
                        st['ps'] = pms.tile([128, 512], F32, tag="ms",
                                            name=f"qk{c}_{rt}")
                    nc.tensor.matmul(st['ps'][:],
                                     wqk_t[f][:, rt * 128:(rt + 1) * 128],
                                     xw[f][:], start=(f == 0), stop=(f == 7))
                    if f == 7:
                        nc.vector.tensor_scalar_add(
                            qkT[rt][:, c * 512:(c + 1) * 512], st['ps'][:],
                            bqk_t[rt][:])
                return _t
            for f in range(8):
                yield _mk(rt, f, st)
        # v natural t-tiles
        for tt in range(4):
            jt = 4 * c + tt
            st = {}
            def _mk(tt, jt, f, st):
                def _t():
                    if f == 0:
                        st['ps'] = pms.tile([128, 256], F32, tag="ms",
                                            name=f"v{c}_{tt}")
                    nc.tensor.matmul(st['ps'][:],
                                     xw[f][:, tt * 128:(tt + 1) * 128],
                                     wv_t[f][:], start=(f == 0), stop=(f == 7))
                    if f == 7:
                        ps = st['ps']
                        v3 = v_t[jt][:].rearrange("p (g e) -> p g e", e=65)
                        nc.vector.tensor_scalar(
                            out=v3[:, :, 64:65],
                            in0=ps[:, 0:4].rearrange("p (g e) -> p g e", e=1),
                            scalar1=0.0, scalar2=1.0, op0=MULT, op1=ADD)
                        nc.vector.tensor_copy(
                            out=v3[:, :, 0:64],
                            in_=ps[:].rearrange("p (g e) -> p g e", e=64))
                return _t
            for f in range(8):
                yield _mk(tt, jt, f, st)

    def yproj_tasks(c):
        for tt in range(4):
            t0 = c * 512 + tt * 128
            st = {}
            for ec in range(2):
                def _mk(t0, ec, k, st):
                    def _t():
                        if ec == 0 and k == 0:
                            st['y'] = sm.tile([128, 1024], F32, tag="y",
                                              name=f"y{t0}", bufs=3)
                        if k == 0:
                            st['ps'] = pms.tile([128, 512], F32, tag="ms",
                                                name=f"yp{t0}_{ec}")
                        nc.tensor.matmul(
                            st['ps'][:], aoT[k][:, t0:t0 + 128],
                            wo_t[k][:, ec * 512:(ec + 1) * 512],
                            start=(k == 0), stop=(k == 1))
                        if k == 1:
                            nc.vector.tensor_copy(
                                out=st['y'][:, ec * 512:(ec + 1) * 512],
                                in_=st['ps'][:])
                            if ec == 1:
                                nc.sync.dma_start(out=y[t0:t0 + 128, :],
                                                  in_=st['y'][:])
                    return _t
                for k in range(2):
                    yield _mk(t0, ec, k, st)

    def start_x_window(c):
        xw = [xp.tile([128, 512], F32R, tag="xw", name=f"xw{c}_{f}")
              for f in range(8)]
        for f in range(8):
            nc.sync.dma_start(
                out=xw[f][:],
                in_=xT[f * 128:(f + 1) * 128, c * 512:(c + 1) * 512])
        return xw

    def emit_pv(pr, jt, p_entry, ao_e, ao_o, first, last):
        p, i0 = p_entry
        for g, ao in ((2 * pr, ao_e), (2 * pr + 1, ao_o)):
            nc.tensor.matmul(
                ao[:, i0:512], v_t[jt][:, g * 65:g * 65 + 65],
                p[:, (g % 2) * 512 + i0:(g % 2) * 512 + 512],
                start=first, stop=last)

    # ---- main schedule ----------------------------------------------------
    rscratch_t = nc.dram_tensor("rscratch", [16, 1, 512], F32)
    rscratch = [rscratch_t.ap()[i] for i in range(16)]
    fill = deque()
    xw0 = start_x_window(0)
    for t in qkv_tasks(0, xw0):
        t()

    for c in range(NT):
        if c + 1 < NT:
            xw_next = start_x_window(c + 1)
            fill.extend(qkv_tasks(c + 1, xw_next))
        if c >= 1:
            fill.extend(yproj_tasks(c - 1))

        njt = 4 * c + 4
        nblocks = 2 * njt
        blk = 0

        def drain_fillers(blocks_left):
            want = -(-len(fill) // max(blocks_left, 1))  # ceil
            for _ in range(min(want, len(fill))):
                fill.popleft()()

        for pr in range(2):
            qt, kt = qkT[pr], qkT[2 + pr]
            ao_e = pao.tile([65, 512], F32, tag="ao", name=f"aoe{c}_{pr}")
            ao_o = pao.tile([65, 512], F32, tag="ao", name=f"aoo{c}_{pr}")
            plist = []
            for jt in range(njt):
                # diagonal blocks: columns i < i0 are fully masked, skip them
                d = jt - 4 * c
                i0 = 128 * d if d >= 1 else 0
                w = 512 - i0
                sc = psc.tile([128, 1024], F32, tag="sc",
                              name=f"sc{c}_{pr}_{jt}")
                nc.tensor.matmul(sc[:, i0:512],
                                 kt[0:64, jt * 128:(jt + 1) * 128],
                                 qt[0:64, c * 512 + i0:(c + 1) * 512],
                                 start=True, stop=True)
                nc.tensor.matmul(sc[:, 512 + i0:1024],
                                 kt[64:128, jt * 128:(jt + 1) * 128],
                                 qt[64:128, c * 512 + i0:(c + 1) * 512],
                                 start=True, stop=True)
                p = pp.tile([128, 1024], att_dt, tag="p", name=f"p{c}_{pr}_{jt}")
                p3 = p[:].rearrange("p (h i) -> p h i", i=512)[:, :, i0:512]
                sc3 = sc[:].rearrange("p (h i) -> p h i", i=512)[:, :, i0:512]
                nc.scalar.activation(p3, sc3, EXP, scale=SCALE)
                if d >= 0:
                    nc.gpsimd.affine_select(
                        out=p3, in_=p3, compare_op=IS_GE, fill=0.0,
                        base=0, channel_multiplier=-1,
                        pattern=[[0, 2], [1, w]])
                plist.append((p, i0))
                if jt >= 1:
                    emit_pv(pr, jt - 1, plist[jt - 1], ao_e, ao_o,
                            first=(jt - 1 == 0), last=False)
                blk += 1
                drain_fillers(nblocks - blk)
            emit_pv(pr, njt - 1, plist[njt - 1], ao_e, ao_o,
                    first=(njt == 1), last=True)

            # normalization: copy PSUM out fast (frees the ao banks); the
            # reciprocal+broadcast+mul tail is deferred into the next chunk's
            # filler stream so it never gates this pipeline.
            for g, ao in ((2 * pr, ao_e), (2 * pr + 1, ao_o)):
                t = sm.tile([65, 512], F32, tag="aosb", name=f"aosb{c}_{g}")
                nc.vector.tensor_copy(out=t[:], in_=ao[:])

                def _norm_tail(c=c, pr=pr, g=g, t=t):
                    r = sm.tile([1, 512], F32, tag="r", name=f"r{c}_{g}", bufs=2)
                    if USE_RECIP_FAST:
                        # custom-DVE ops don't handle nonzero partition
                        # offsets; stage the sums row at partition 0 first.
                        s_row = sm.tile([1, 512], F32, tag="srow",
                                        name=f"srow{c}_{g}", bufs=2)
                        nc.vector.tensor_copy(out=s_row[:], in_=t[64:65, :])
                        nc.vector.reciprocal_approx_fast(out=r[:],
                                                         in_=s_row[:])
                    else:
                        nc.vector.reciprocal(r[:], t[64:65, :])
                    # broadcast R across 64 partitions via a DRAM bounce
                    # (keeps gpsimd free for the causal-mask selects; an
                    # SBUF-source broadcast AP is not expressible).
                    rd = rscratch[4 * c + g]
                    nc.sync.dma_start(out=rd, in_=r[:])
                    rb = sm.tile([64, 512], F32, tag="rb", name=f"rb{c}_{g}", bufs=2)
                    nc.sync.dma_start(out=rb[:],
                                      in_=rd.to_broadcast([64, 512]))
                    nc.vector.tensor_mul(
                        aoT[pr][(g % 2) * 64:(g % 2) * 64 + 64,
                                c * 512:(c + 1) * 512],
                        t[0:64, :], rb[:])
                fill.append(_norm_tail)

    while fill:
        fill.popleft()()
    for t in yproj_tasks(NT - 1):
        t()


_CACHE = {}


def _build():
    if 'nc' in _CACHE:
        return _CACHE['nc']
    nc = bacc.Bacc("TRN2", target_bir_lowering=False, debug=False)
    dram = {
        'xT': nc.dram_tensor("xT", [D, N], F32R, kind="ExternalInput").ap(),
        'wqk': nc.dram_tensor("wqk", [D, 512], F32R, kind="ExternalInput").ap(),
        'wv': nc.dram_tensor("wv", [D, 256], F32R, kind="ExternalInput").ap(),
        'wo': nc.dram_tensor("wo", [256, D], F32R, kind="ExternalInput").ap(),
        'bqk': nc.dram_tensor("bqk", [512, 1], F32, kind="ExternalInput").ap(),
        'y': nc.dram_tensor("y", [N, D], F32, kind="ExternalOutput").ap(),
    }
    from contextlib import ExitStack
    with tile.TileContext(nc) as tc, ExitStack() as ctx:
        _emit(ctx, nc, tc, dram)
    nc.compile()
    _CACHE['nc'] = nc
    return nc


def _prep_core_inputs(x, W_qkv, b_qkv, W_out, core):
    b = core // 4
    h0 = HPC * (core % 4)
    r0 = HD * h0
    q_rows = W_qkv[r0:r0 + 256]
    k_rows = W_qkv[D + r0:D + r0 + 256]
    v_rows = W_qkv[2 * D + r0:2 * D + r0 + 256]
    return {
        'xT': np.ascontiguousarray(x[b].T),
        'wqk': np.ascontiguousarray(np.concatenate([q_rows, k_rows], 0).T),
        'wv': np.ascontiguousarray(v_rows.T),
        'wo': np.ascontiguousarray(W_out[:, r0:r0 + 256].T),
        'bqk': np.concatenate(
            [b_qkv[r0:r0 + 256], b_qkv[D + r0:D + r0 + 256]]).reshape(512, 1),
    }


def kernel(x, W_qkv, b_qkv, W_out, b_out, _trace=False, _tmpdir=None):
    x = np.asarray(x, dtype=np.float32)
    W_qkv = np.asarray(W_qkv, dtype=np.float32)
    b_qkv = np.asarray(b_qkv, dtype=np.float32)
    W_out = np.asarray(W_out, dtype=np.float32)
    b_out = np.asarray(b_out, dtype=np.float32)

    in_maps = [_prep_core_inputs(x, W_qkv, b_qkv, W_out, c)
               for c in range(NCORES)]
    nc = _build()
    res = run_bass_kernel_spmd(nc, in_maps, list(range(NCORES)),
                               trace=_trace, tmpdir=_tmpdir)

    # v-bias contribution (softmax rows sum to 1) + output bias, as one
    # constant vector added on the host.
    bv = b_qkv[2 * D:3 * D]
    const = (b_out.astype(np.float64)
             + W_out.astype(np.float64) @ bv.astype(np.float64))
    out = np.empty((B, N, D), dtype=np.float32)
    for b in range(B):
        acc = np.zeros((N, D), dtype=np.float64)
        for g in range(4):
            acc += res.results[4 * b + g]['y'].astype(np.float64)
        out[b] = (acc + const).astype(np.float32)
    if _trace:
        kernel.last_exec_time_ns = res.exec_time_ns
        kernel.last_trace = (res.instructions_and_trace[1]
                             if res.instructions_and_trace else None)
    return out
